# revision 19
# baseline (speedup 1.0000x reference)
"""DglGraphConvolution Trainium2 kernel — dense block-adjacency SpMM.

Key idea: segment_sum over edges == A @ x where A[d, s] = multiplicity of
edge (s -> d). Host re-encodes the edge index lists as the dense count
matrix A^T (src-major, fp8 e4m3: counts are tiny ints, exactly
representable) — pure index preprocessing, no model data touched.

Because aggregation and the feature transform are both linear, the device
aggregates RAW text first and applies W after. Work is pipelined over
dst-halves of 2048 columns so each half's epilogue (psum eviction,
W-transform, bias, store) hides under the next half's accumulation:

  per graph g (2 per core, data-parallel over B=16 on 8 cores):
    per dst-half h (4 psum banks each; the other 4 banks belong to the
    previous half, whose epilogue is still draining):
      stage 1: for ws in 0..31:  (src windows of 128 nodes)
         tagg[fin, d] += text_ws[s, fin]-as-stationary @ A^T[s, d]
         (4 matmuls of 512 moving cols; A^T streamed from HBM as fp8 in
          1 MiB quad-window DMAs alternating between the SP and ACT DGE
          queues; text converted f32->bf16 on DVE; the stationary operand
          is reused across 4 matmuls so LDWEIGHTS pipelines away)
      epilogue (emitted at the start of the NEXT half's stage 1):
         tagg_n = tagg * recip_deg[d]      (DVE, psum -> bf16 sbuf)
         out^T[f, d] = W-as-stationary @ tagg_n[fin, d]   (PE)
         out = out^T + bias[f]             (ACT per-partition bias, bf16)

deg comes from the same index-only host prep (recip = 1/(deg+1), bf16,
replicated across the 128 partitions). Output is written transposed
[f, d] and untransposed on the host.
"""

import numpy as np

B, N, E, F = 16, 4096, 131072, 128
NCORES = 8
GPC = B // NCORES  # graphs per core
W = 128  # src window (partition) size
NW = N // W  # 32 src windows
H = 2  # dst halves
NH = N // H  # 2048 dst per half
QH = NH // 512  # 4 psum banks (512-wide matmuls) per half
PW = 4  # src windows per A DMA (1 MiB quads)
NP = NW // PW  # 8 quads per half
QW = 512  # moving free-dim per matmul

_cache = {}


def _build_program():
    from contextlib import ExitStack

    import concourse.bacc as bacc
    import concourse.tile as tile
    from concourse import mybir
    from concourse._compat import get_trn_type

    f32 = mybir.dt.float32
    bf16 = mybir.dt.bfloat16
    fp8 = mybir.dt.float8e4

    nc = bacc.Bacc(get_trn_type() or "TRN2", target_bir_lowering=False, debug=False)

    text_d = nc.dram_tensor("text", [GPC, N, F], f32, kind="ExternalInput")
    a_d = nc.dram_tensor(
        "acnt", [GPC, H, NP, W, PW, NH], fp8, kind="ExternalInput"
    )
    rec_d = nc.dram_tensor("recrep", [GPC, W, N], bf16, kind="ExternalInput")
    w_d = nc.dram_tensor("weight", [F, F], f32, kind="ExternalInput")
    bias_d = nc.dram_tensor("biascol", [F, 1], f32, kind="ExternalInput")
    out_d = nc.dram_tensor("out", [GPC, F, N], bf16, kind="ExternalOutput")

    with tile.TileContext(nc) as tc, ExitStack() as ctx:
        const = ctx.enter_context(tc.tile_pool(name="const", bufs=1))
        tpool = ctx.enter_context(tc.tile_pool(name="tpool", bufs=4))
        spool = ctx.enter_context(tc.tile_pool(name="spool", bufs=2))
        apool = ctx.enter_context(tc.tile_pool(name="apool", bufs=4))
        gpool = ctx.enter_context(tc.tile_pool(name="gpool", bufs=2))
        opool = ctx.enter_context(tc.tile_pool(name="opool", bufs=4))
        psum = ctx.enter_context(tc.tile_pool(name="psum", bufs=8, space="PSUM"))

        # const DMAs are emitted inside the loop (at g0/h0/p1) so they do
        # not sit ahead of the first A rows in the cold DGE queues
        w_sb = const.tile([F, F], f32)
        w_bf = const.tile([F, F], bf16)
        bias_sb = const.tile([F, 1], f32)

        def emit_epilogue(g, h, accs, recrep):
            tagg = gpool.tile([F, NH], bf16, tag="tagg", name=f"tagg{g}_{h}")
            for q in range(QH):
                nc.vector.tensor_tensor(
                    out=tagg[:, QW * q : QW * (q + 1)],
                    in0=accs[q][:],
                    in1=recrep[:, h * NH + QW * q : h * NH + QW * (q + 1)],
                    op=mybir.AluOpType.mult,
                )
            for q in range(QH):
                o_ps = psum.tile([F, QW], f32, tag="acc", name=f"ops{g}_{h}_{q}")
                nc.tensor.matmul(
                    out=o_ps[:],
                    lhsT=w_bf[:],
                    rhs=tagg[:, QW * q : QW * (q + 1)],
                    start=True,
                    stop=True,
                )
                obf = opool.tile([F, QW], bf16, tag="o")
                nc.scalar.activation(
                    obf[:],
                    o_ps[:],
                    mybir.ActivationFunctionType.Identity,
                    bias=bias_sb[:, 0:1],
                )
                nc.sync.dma_start(
                    out_d[g, :, h * NH + QW * q : h * NH + QW * (q + 1)], obf[:]
                )

        pending = None  # (g, h, accs, recrep) of the half awaiting epilogue
        nq = 0  # global A-DMA parity for queue alternation
        recreps = {}
        st_alls = {}

        def load_text(gt, ws, engine):
            tt = tpool.tile([W, F], f32, tag="t")
            engine.dma_start(tt[:], text_d[gt, W * ws : W * (ws + 1), :])
            nc.vector.tensor_copy(st_alls[gt][:, F * ws : F * (ws + 1)], tt[:])

        for g in range(GPC):
            for h in range(H):
                half_accs = None
                for p in range(NP):
                    ramp = g == 0 and h == 0 and p < 2
                    if ramp:
                        # cold-start: per-row tiles and row-sized DMAs so the
                        # first matmuls only wait on their own 256 KiB row
                        rows = []
                        for j in range(PW):
                            ws = PW * p + j
                            arj = apool.tile(
                                [W, NH], fp8, tag="a0", bufs=8, name=f"ar{ws}"
                            )
                            (nc.sync if ws % 2 == 0 else nc.scalar).dma_start(
                                arj[:], a_d[g, h, p, :, j, :]
                            )
                            if g == 0 and h == 0:
                                if p == 0 and j == 0:
                                    recreps[0] = gpool.tile(
                                        [W, N], bf16, tag="rec", name="rec0"
                                    )
                                    st_alls[0] = spool.tile(
                                        [W, NW * F], bf16, tag="s", name="st0"
                                    )
                                load_text(0, ws, nc.sync if ws % 2 else nc.scalar)
                            rows.append(arj)
                    else:
                        ar = apool.tile(
                            [W, PW, NH], fp8, tag="a", name=f"a{g}_{h}_{p}"
                        )
                        (nc.sync if nq % 2 == 0 else nc.scalar).dma_start(
                            ar[:], a_d[g, h, p]
                        )
                    nq += 1
                    if g == 0 and h == 0 and p == 1:
                        nc.sync.dma_start(w_sb[:], w_d[:, :])
                        nc.vector.tensor_copy(w_bf[:], w_sb[:])
                        nc.scalar.dma_start(bias_sb[:], bias_d[:, :])
                    if h == 0 and p == 4:
                        # deferred: not needed until psum eviction
                        nc.scalar.dma_start(recreps[g][:], rec_d[g])
                    if p == 0:
                        # fresh banks for this half (the previous half's
                        # banks drain through its epilogue below)
                        half_accs = [
                            psum.tile([W, QW], f32, tag="acc", name=f"acc{g}{h}{q}")
                            for q in range(QH)
                        ]
                    for j in range(PW):
                        ws = PW * p + j
                        if g == 0 and h == 0 and not ramp:
                            load_text(0, ws, nc.sync)
                        if h == 1 and g + 1 < GPC:
                            # prefetch next graph's text under this half
                            if p == 0 and j == 0:
                                recreps[g + 1] = gpool.tile(
                                    [W, N], bf16, tag="rec", name=f"rec{g + 1}"
                                )
                                st_alls[g + 1] = spool.tile(
                                    [W, NW * F], bf16, tag="s", name=f"st{g + 1}"
                                )
                            load_text(g + 1, ws, nc.sync)
                        rhs = rows[j][:, :] if ramp else ar[:, j, :]
                        for q in range(QH):
                            nc.tensor.matmul(
                                out=half_accs[q][:],
                                lhsT=st_alls[g][:, F * ws : F * (ws + 1)],
                                rhs=rhs[:, QW * q : QW * (q + 1)],
                                start=(ws == 0),
                                stop=(ws == NW - 1),
                            )
                    if p == 0 and pending is not None:
                        # previous half's epilogue drains under this stage 1
                        emit_epilogue(*pending)
                        pending = None
                pending = (g, h, half_accs, recreps[g])
        emit_epilogue(*pending)

    nc.compile()
    return nc


def _prep_graph(src, dst):
    """Index-only: dense src-major count matrix [NW, W, N] (float32 counts)
    and the replicated reciprocal degree row [W, N]."""
    lin = src.astype(np.int64) * N + dst
    cnt = np.bincount(lin, minlength=N * N).astype(np.float32)
    assert cnt.max() <= 16, f"edge multiplicity overflow: {cnt.max()}"
    deg = np.bincount(dst, minlength=N).astype(np.float32)
    rec = (1.0 / (deg + 1.0)).astype(np.float32)
    recrep = np.ascontiguousarray(np.broadcast_to(rec[None, :], (W, N)))
    return cnt.reshape(NW, W, N), recrep


def kernel(text, weight, bias, edge_src, edge_dst):
    import ml_dtypes

    text = np.asarray(text, dtype=np.float32)
    weight = np.asarray(weight, dtype=np.float32)
    bias = np.asarray(bias, dtype=np.float32)
    edge_src = np.asarray(edge_src, dtype=np.int32)
    edge_dst = np.asarray(edge_dst, dtype=np.int32)

    if "nc" not in _cache:
        _cache["nc"] = _build_program()
    nc = _cache["nc"]

    in_maps = []
    for k in range(NCORES):
        acnt = np.empty((GPC, H, NP, W, PW, NH), dtype=ml_dtypes.float8_e4m3)
        recrep = np.empty((GPC, W, N), dtype=ml_dtypes.bfloat16)
        for g in range(GPC):
            b = k * GPC + g
            cnt, rr = _prep_graph(edge_src[b], edge_dst[b])
            # [NW, W, N] -> [H, NP, W, PW, NH]
            acnt[g] = (
                cnt.reshape(NP, PW, W, H, NH)
                .transpose(3, 0, 2, 1, 4)
                .astype(ml_dtypes.float8_e4m3)
            )
            recrep[g] = rr
        in_maps.append(
            {
                "text": text[k * GPC : (k + 1) * GPC],
                "acnt": acnt,
                "recrep": recrep,
                "weight": weight,
                "biascol": bias.reshape(F, 1),
            }
        )

    _cache["in_maps"] = in_maps

    from concourse.bass_utils import run_bass_kernel_spmd

    res = run_bass_kernel_spmd(nc, in_maps, list(range(NCORES)))
    out = np.concatenate(
        [
            np.asarray(res.results[k]["out"])
            .astype(np.float32)
            .transpose(0, 2, 1)
            for k in range(NCORES)
        ],
        axis=0,
    )
    return out
